# revision 12
# baseline (speedup 1.0000x reference)
"""ContrastiveHead loss kernel for 8 Trainium2 NeuronCores.

Data-parallel shard of B across 8 cores; each core MLPs its 1024 rows
(transposed layout: features on partitions, rows on the free dim; fp8
DoubleRow matmuls), normalizes the [E=128, 1024] features, all-gathers
bf16 features, then computes its [1024, 8192] block of the similarity
matrix and the masked logsumexp sums. Per-row exp-sums plus the raw
bf16 features go back to the host, which recomputes the self/pos
similarities exactly and finishes the loss in f64.

Schedule (all on one PE stream, emission order == execution order):
- biases first, then x/w0/w1/w2 DMAs in first-use order, coarse slabs
  across 4 posting queues.
- warmup 2KB AllGather right away so the CC engine's ~10us first-use
  startup cost is paid during the DMA ramp.
- L0h0 -> L1h0 -> L2h0+norm -> gather A (rows 0:512, Shared output)
  issued at ~1/2 of the MLP, fully hidden under the h1 half.
- L0h1 -> L1h1 with the 8 sim chunks over gather-A columns x h0 rows
  interleaved between groups; their exp runs as Schraudolph int16
  (bf16-bitcast) on the otherwise idle Vector engine, keeping the
  Scalar engine on the Identity/Sqrt table until the tail.
- L2h1+norm -> gathers Q3/Q4 (rows 512:768, 768:1024).
- tail: remaining 24 sim chunks; exp split ACT (Exp+accum_out, ~15) /
  DVE (Schraudolph, ~9) to finish together.
- logsumexp via the constant bound max=1 (normalized rows): lse = 1/T +
  log(sum_j exp((S_ij-1)/T)); self term excluded on the host using the
  shipped bf16 features (exact same values the sim matmul consumed).
"""

import os
import sys

for _p in ("/opt/trn_rl_repo",):
    if os.path.isdir(_p) and _p not in sys.path:
        sys.path.append(_p)

import ml_dtypes
import numpy as np

import concourse.bass as bass
import concourse.mybir as mybir
import concourse.tile as tile
from concourse import bacc
from concourse.bass_utils import run_bass_kernel_spmd

BF16 = ml_dtypes.bfloat16
F32 = mybir.dt.float32
BF = mybir.dt.bfloat16
F8 = mybir.dt.float8e4
FP8 = mybir.dt.np(F8)

B, D, H, E = 4096, 2048, 2048, 128
T = 0.07
SCALE = float(1.0 / T)
NCORES = 8
BS = B // NCORES          # rows per view per core (512)
M = 2 * BS                # local feature rows (1024)
HM = M // 2               # rows per pipeline half (512)
KT = D // 128             # 16 contraction tiles for D/H
NT = H // 128             # 16 output-feature tiles for hidden layers
MT = M // 128              # 8 local row tiles
NG = NCORES * M            # 8192 gathered rows
CHUNK = 2048               # sim free-dim chunk (4-bank PSUM tile)
NCHUNK = NG // CHUNK       # 4 sim chunks per row tile

# Schraudolph integer exp, int16/bf16 variant: the bf16 bit pattern of
# exp(x) is approximately int16(A16*x + B16); bf16 output gets the DVE
# 2x reduce rate. B16 offset tuned for minimal loss bias.
SCH_A16 = float(2.0**23 / np.log(2.0) / 65536.0)
SCH_MUL = SCH_A16 * SCALE
SCH_ADD = float((16256.0 - 4.0) - SCH_A16 * SCALE)


def _build():
    nc = bacc.Bacc(num_devices=NCORES)

    x = nc.dram_tensor("x", [2, 128, KT, HM], F8, kind="ExternalInput")
    w0 = nc.dram_tensor("w0", [NT, 128, KT, 128], F8, kind="ExternalInput")
    w1 = nc.dram_tensor("w1", [NT, 128, KT, 128], F8, kind="ExternalInput")
    w2 = nc.dram_tensor("w2", [128, KT, 128], F8, kind="ExternalInput")
    ball = nc.dram_tensor("ball", [128, 2 * NT + 1], F32, kind="ExternalInput")
    out = nc.dram_tensor("out", [128, MT], F32, kind="ExternalOutput")
    ftout = nc.dram_tensor("ftout", [128, M], BF, kind="ExternalOutput")

    AF = mybir.ActivationFunctionType
    MULT = mybir.AluOpType.mult
    ADD = mybir.AluOpType.add
    DR = mybir.MatmulPerfMode.DoubleRow
    groups = [list(range(NCORES))]

    with tile.TileContext(nc) as tc:
        with (
            tc.tile_pool(name="singles", bufs=1) as singles,
            tc.tile_pool(name="esc", bufs=3) as esc,
            tc.tile_pool(name="pmm", bufs=4, space="PSUM") as pmm,
            tc.tile_pool(name="dram", bufs=1, space="DRAM") as dram,
        ):
            # ---- SBUF tiles ----
            w0s = singles.tile([128, NT, KT, 128], F8)
            w1s = singles.tile([128, NT, KT, 128], F8)
            wsl2 = singles.tile([128, KT, 128], F8)
            xt = singles.tile([128, 2, KT, HM], F8)
            balls = singles.tile([128, 2 * NT + 1], F32)
            h0 = singles.tile([128, NT, M], F8)
            h1 = singles.tile([128, NT, M], F8)
            eT = singles.tile([128, M], F32)
            sq = singles.tile([128, M], BF)
            rnorm = singles.tile([128, M], F32)
            rrec = singles.tile([128, M], F32)
            fT = singles.tile([128, M], BF)
            FT = singles.tile([128, NG], BF)
            ones = singles.tile([128, 128], BF)
            nbias = singles.tile([128, 1], F32)
            sums = singles.tile([128, MT, 2 * NCHUNK], F32)
            stot = singles.tile([128, MT], F32)

            # ---- input DMAs, first-use order across 4 posting queues ----
            def wslab(dst, src, lo, hi, eng):
                eng.dma_start(
                    out=dst[:, lo:hi, :, :],
                    in_=src[lo:hi].rearrange("t p k n -> p t k n"),
                )

            # The sync ring carries the collective-critical small DMAs plus
            # an early share of the input stream (all input items complete
            # by ~40us, well before cc_inA is needed); the scalar ring takes
            # the rest of the inputs. The gpsimd ring carries ONLY the
            # collectives so their triggers are never queued behind bulk
            # DMA issues.
            warm_in = dram.tile([128, 256], BF, name="warm_in")
            warm_out = dram.tile(
                [NCORES * 128, 256], BF, name="warm_out", addr_space="Shared"
            )
            nc.scalar.dma_start(out=warm_in, in_=x[1, :, 0:1, :].bitcast(BF))
            nc.gpsimd.collective_compute(
                "AllGather", mybir.AluOpType.bypass, replica_groups=groups,
                ins=[warm_in.opt()], outs=[warm_out.opt()],
            )

            nc.sync.dma_start(out=balls, in_=ball[:, :])
            nc.sync.dma_start(out=xt[:, 0, 0:4, :], in_=x[0, :, 0:4, :])
            wslab(w0s, w0, 0, 1, nc.scalar)
            nc.sync.dma_start(out=xt[:, 0, 4:8, :], in_=x[0, :, 4:8, :])
            nc.scalar.dma_start(out=xt[:, 0, 8:KT, :], in_=x[0, :, 8:KT, :])
            wslab(w0s, w0, 1, 5, nc.scalar)
            wslab(w0s, w0, 5, 10, nc.sync)
            wslab(w0s, w0, 10, NT, nc.scalar)
            wslab(w1s, w1, 0, 5, nc.sync)
            wslab(w1s, w1, 5, NT, nc.scalar)
            nc.sync.dma_start(out=xt[:, 1, :, :], in_=x[1, :, :, :])
            nc.scalar.dma_start(out=wsl2, in_=w2[:, :, :])

            nc.vector.memset(ones, 1.0)
            nc.vector.memset(nbias, -SCALE)

            def hidden_layer(src_ap, dst, wt, boff, func, hsl, extra=None):
                """fp8 DoubleRow layer on one 512-row half; 2 tn chains per
                [128,1024] PSUM tile (2 banks), 2 512-col ACT drains. extra:
                dict group_idx -> list of callbacks emitted after that
                group."""
                width = hsl.stop - hsl.start
                tpg = 1024 // width  # tn chains per [128,1024] psum tile
                for gi, tn0 in enumerate(range(0, NT, tpg)):
                    ps = pmm.tile([128, 1024], F32, tag="mm")
                    for j in range(tpg):
                        tn = tn0 + j
                        for tk in range(0, KT, 2):
                            nc.tensor.matmul(
                                ps[:, j * width : (j + 1) * width],
                                lhsT=wt[:, tn, tk : tk + 2, :],
                                rhs=src_ap(tk, hsl),
                                start=(tk == 0),
                                stop=(tk == KT - 2),
                                perf_mode=DR,
                            )
                    for j in range(tpg):
                        tn = tn0 + j
                        nc.scalar.activation(
                            out=dst[:, tn, hsl],
                            in_=ps[:, j * width : (j + 1) * width],
                            func=func,
                            bias=balls[:, boff + tn : boff + tn + 1],
                            scale=1.0,
                        )
                    if extra and gi in extra:
                        for cb in extra[gi]:
                            cb()

            def l2_norm(h, hsl):
                """L2 layer + normalize rows hsl -> fT[:, hsl] (bf16)."""
                width = hsl.stop - hsl.start
                ps2 = pmm.tile([128, 1024], F32, tag="mm")
                for tk in range(0, KT, 2):
                    nc.tensor.matmul(
                        ps2[:, 0:width],
                        lhsT=wsl2[:, tk : tk + 2, :],
                        rhs=h[:, tk : tk + 2, hsl],
                        start=(tk == 0),
                        stop=(tk == KT - 2),
                        perf_mode=DR,
                    )
                nc.scalar.activation(
                    out=eT[:, hsl], in_=ps2[:, 0:width], func=AF.Identity,
                    bias=balls[:, 2 * NT : 2 * NT + 1], scale=1.0,
                )
                nc.scalar.activation(
                    out=sq[:, hsl], in_=eT[:, hsl], func=AF.Square, scale=1.0,
                )
                nc.tensor.matmul(
                    ps2[:, width : 2 * width], lhsT=ones, rhs=sq[:, hsl],
                    start=True, stop=True,
                )
                nc.scalar.activation(
                    out=rnorm[:, hsl], in_=ps2[:, width : 2 * width],
                    func=AF.Sqrt, scale=1.0,
                )
                nc.vector.reciprocal_approx_fast(
                    out=rrec[:, hsl], in_=rnorm[:, hsl]
                )
                nc.vector.tensor_mul(fT[:, hsl], eT[:, hsl], rrec[:, hsl])

            def gather(hsl, ft_off, tag):
                """all-gather fT[:, hsl]; blocks land FT[:, ft_off:...]."""
                width = hsl.stop - hsl.start
                cc_in = dram.tile([128, width], BF, name=f"cc_in{tag}")
                cc_out = dram.tile(
                    [NCORES * 128, width], BF, name=f"cc_out{tag}",
                    addr_space="Shared",
                )
                nc.sync.dma_start(out=cc_in, in_=fT[:, hsl])
                nc.gpsimd.collective_compute(
                    "AllGather", mybir.AluOpType.bypass, replica_groups=groups,
                    ins=[cc_in.opt()], outs=[cc_out.opt()],
                )
                half = NCORES // 2
                nc.sync.dma_start(
                    out=FT[:, ft_off : ft_off + half * width],
                    in_=cc_out[0 : half * 128, :].rearrange(
                        "(r p) w -> p r w", r=half
                    ),
                )
                nc.sync.dma_start(
                    out=FT[:, ft_off + half * width : ft_off + NCORES * width],
                    in_=cc_out[half * 128 :, :].rearrange(
                        "(r p) w -> p r w", r=half
                    ),
                )

            def sim_chunk(c, m, eng):
                """sim rows m-tile x FT columns chunk c (2048 cols as two
                [128,1024] PSUM halves); exp-sum on eng."""
                lhs = fT[:, m * 128 : (m + 1) * 128]
                for h in range(2):
                    ps = pmm.tile([128, 1024], F32, tag="mm")
                    for q in range(2):
                        j0 = c * CHUNK + h * 1024 + q * 512
                        nc.tensor.matmul(
                            ps[:, q * 512 : (q + 1) * 512],
                            lhsT=lhs, rhs=FT[:, j0 : j0 + 512],
                            start=True, stop=True,
                        )
                    slot = sums[:, m, 2 * c + h : 2 * c + h + 1]
                    if eng == "dve":
                        sch = esc.tile([128, 1024], mybir.dt.int16, tag="sch")
                        nc.vector.tensor_scalar(
                            out=sch, in0=ps, scalar1=SCH_MUL, scalar2=SCH_ADD,
                            op0=MULT, op1=ADD,
                        )
                        nc.vector.reduce_sum(
                            slot, sch.bitcast(BF), axis=mybir.AxisListType.X,
                        )
                    else:
                        escr = esc.tile([128, 1024], BF, tag="escr")
                        nc.scalar.activation(
                            out=escr, in_=ps, func=AF.Exp, scale=SCALE,
                            bias=nbias, accum_out=slot,
                        )

            xsrc = lambda h: (lambda tk, hsl: xt[:, h, tk : tk + 2, :])
            hsrc = lambda hh: (lambda tk, hsl: hh[:, tk : tk + 2, hsl])

            # ---- first half: rows 0:512; gather A as two 64KB pieces so
            # the c0 columns land ~12us before c1 ----
            h0sl = slice(0, HM)
            h1sl = slice(HM, M)
            q3sl = slice(HM, HM + HM // 2)
            q4sl = slice(HM + HM // 2, M)
            hidden_layer(xsrc(0), h0, w0s, 0, AF.Relu, h0sl)
            hidden_layer(hsrc(h0), h1, w1s, NT, AF.Identity, h0sl)
            l2_norm(h1, h0sl)
            gather(slice(0, HM // 2), 0, "A1")
            gather(slice(HM // 2, HM), NG // 4, "A2")

            # ---- second half; L1 by 256-row quarters so gather Q3 launches
            # ~20us earlier and its mesh overlaps the L1-q4 compute. The 12
            # pre-tail sim chunks all run Schraudolph on the DVE so the
            # Scalar engine keeps the Identity/Sqrt table until the tail.
            early = [(0, 0), (0, 1), (0, 2), (0, 3),
                     (1, 0), (1, 1), (1, 2), (1, 3),
                     (0, 4), (1, 4)]
            ei = iter(early)

            def emit_early(k):
                def cb():
                    for _ in range(k):
                        cm = next(ei, None)
                        if cm is not None:
                            sim_chunk(cm[0], cm[1], "dve")
                return cb

            hidden_layer(
                xsrc(1), h0, w0s, 0, AF.Relu, h1sl,
                extra={gi: [emit_early(1)] for gi in (4, 5, 6, 7)},
            )
            hidden_layer(
                hsrc(h0), h1, w1s, NT, AF.Identity, q3sl,
                extra={gi: [emit_early(1)] for gi in range(4)},
            )
            l2_norm(h1, q3sl)
            gather(q3sl, NG // 2, "Q3")
            hidden_layer(
                hsrc(h0), h1, w1s, NT, AF.Identity, q4sl,
                extra={1: [emit_early(1)], 3: [emit_early(1)]},
            )
            l2_norm(h1, q4sl)
            gather(q4sl, 3 * NG // 4, "Q4")

            # ---- tail sim chunks: ACT/DVE split ~3:2, ordered by when
            # their inputs become available ----
            tail = [(0, 5), (1, 5)]
            tail += [(2, m) for m in range(4)]
            tail += [(c, m) for m in (6, 7) for c in range(2)]
            tail += [(2, m) for m in range(4, MT)]
            tail += [(3, m) for m in range(MT)]
            for i, (c, m) in enumerate(tail):
                sim_chunk(c, m, "dve" if i % 5 in (2, 4) else "act")

            nc.vector.reduce_sum(stot, sums, axis=mybir.AxisListType.X)
            nc.sync.dma_start(out=ftout[:, 0:HM], in_=fT[:, h0sl])
            nc.sync.dma_start(out=ftout[:, HM:M], in_=fT[:, h1sl])
            nc.sync.dma_start(out=out[:, :], in_=stot)

    nc.finalize()
    return nc


_NC_CACHE = None


def _get_nc():
    global _NC_CACHE
    if _NC_CACHE is None:
        _NC_CACHE = _build()
    return _NC_CACHE


def host_reduce(stot_rows, f):
    """Per-core loss sum from device exp-sums and bf16 features.

    stot_rows: [M] f64 row sums of exp((S-1)/T) incl. the self term.
    f: [M, E] f64 (bf16-rounded features, exactly what the matmul used).
    """
    dself = (f * f).sum(axis=1)
    dpos = (f[:HM] * f[HM:]).sum(axis=1)
    dpos = np.concatenate([dpos, dpos])
    sexcl = stot_rows - np.exp(SCALE * dself - SCALE)
    return (np.log(sexcl) + SCALE * (1.0 - dpos)).sum()


def _prep_w(W, ntiles):
    K = W.shape[0]
    kt = K // 128
    arr = W.reshape(kt, 128, ntiles, 128).transpose(2, 1, 0, 3)
    return np.ascontiguousarray(arr.astype(FP8))


def _prep_b(b, ntiles):
    return np.asarray(b, np.float32).reshape(ntiles, 128).T


def kernel(input1, input2, W0, b0, W1, b1, W2, b2):
    input1 = np.asarray(input1, np.float32)
    input2 = np.asarray(input2, np.float32)
    w0p = _prep_w(np.asarray(W0, np.float32), NT)
    w1p = _prep_w(np.asarray(W1, np.float32), NT)
    w2p = _prep_w(np.asarray(W2, np.float32), 1)[0]
    ballp = np.ascontiguousarray(
        np.concatenate(
            [_prep_b(b0, NT), _prep_b(b1, NT),
             np.asarray(b2, np.float32).reshape(128, 1)],
            axis=1,
        )
    )

    in_maps = []
    for r in range(NCORES):
        xr = np.concatenate(
            [input1[r * BS : (r + 1) * BS], input2[r * BS : (r + 1) * BS]],
            axis=0,
        )
        # [2, 128, KT, HM]: half-major, features on partitions
        xp = np.ascontiguousarray(
            xr.reshape(2, HM, KT, 128).transpose(0, 3, 2, 1).astype(FP8)
        )
        in_maps.append(
            {"x": xp, "w0": w0p, "w1": w1p, "w2": w2p, "ball": ballp}
        )

    nc = _get_nc()
    res = run_bass_kernel_spmd(
        nc,
        in_maps,
        core_ids=list(range(NCORES)),
        trace=bool(int(os.environ.get("KERNEL_TRACE", "0"))),
    )
    total = np.float64(0.0)
    for r in range(NCORES):
        stot = np.asarray(res.results[r]["out"], np.float64)  # [128, MT]
        ft = np.asarray(res.results[r]["ftout"]).astype(np.float64)  # [128, M]
        # stot[:, m] is rows m*128..(m+1)*128; flatten to [M]
        stot_rows = stot.T.reshape(M)
        f = ft.T  # [M, E]
        total += host_reduce(stot_rows, f)
    loss = np.float32(total / (2 * B))
    if res.exec_time_ns is not None:
        kernel.last_exec_time_ns = res.exec_time_ns
    return np.asarray(loss, np.float32)


kernel.last_exec_time_ns = None


# revision 13
# speedup vs baseline: 1.1436x; 1.1436x over previous
"""ContrastiveHead loss kernel for 8 Trainium2 NeuronCores.

Data-parallel shard of B across 8 cores; each core MLPs its 1024 rows
(transposed layout: features on partitions, rows on the free dim; fp8
DoubleRow matmuls), normalizes the [E=128, 1024] features, all-gathers
bf16 features, then computes its [1024, 8192] block of the similarity
matrix and the masked logsumexp sums. Per-row exp-sums plus the raw
bf16 features go back to the host, which recomputes the self/pos
similarities exactly and finishes the loss in f64.

Schedule (all on one PE stream, emission order == execution order):
- biases first, then x/w0/w1/w2 DMAs in first-use order, coarse slabs
  across 4 posting queues.
- warmup 2KB AllGather right away so the CC engine's ~10us first-use
  startup cost is paid during the DMA ramp.
- L0h0 -> L1h0 -> L2h0+norm -> gather A (rows 0:512, Shared output)
  issued at ~1/2 of the MLP, fully hidden under the h1 half.
- L0h1 -> L1h1 with the 8 sim chunks over gather-A columns x h0 rows
  interleaved between groups; their exp runs as Schraudolph int16
  (bf16-bitcast) on the otherwise idle Vector engine, keeping the
  Scalar engine on the Identity/Sqrt table until the tail.
- L2h1+norm -> gathers Q3/Q4 (rows 512:768, 768:1024).
- tail: remaining 24 sim chunks; exp split ACT (Exp+accum_out, ~15) /
  DVE (Schraudolph, ~9) to finish together.
- logsumexp via the constant bound max=1 (normalized rows): lse = 1/T +
  log(sum_j exp((S_ij-1)/T)); self term excluded on the host using the
  shipped bf16 features (exact same values the sim matmul consumed).
"""

import os
import sys

for _p in ("/opt/trn_rl_repo",):
    if os.path.isdir(_p) and _p not in sys.path:
        sys.path.append(_p)

import ml_dtypes
import numpy as np

import concourse.bass as bass
import concourse.mybir as mybir
import concourse.tile as tile
from concourse import bacc
from concourse.bass_utils import run_bass_kernel_spmd

BF16 = ml_dtypes.bfloat16
F32 = mybir.dt.float32
BF = mybir.dt.bfloat16
F8 = mybir.dt.float8e4
FP8 = mybir.dt.np(F8)

B, D, H, E = 4096, 2048, 2048, 128
T = 0.07
SCALE = float(1.0 / T)
NCORES = 8
BS = B // NCORES          # rows per view per core (512)
M = 2 * BS                # local feature rows (1024)
HM = M // 2               # rows per pipeline half (512)
KT = D // 128             # 16 contraction tiles for D/H
NT = H // 128             # 16 output-feature tiles for hidden layers
MT = M // 128              # 8 local row tiles
NG = NCORES * M            # 8192 gathered rows
CHUNK = 2048               # sim free-dim chunk (4-bank PSUM tile)
NCHUNK = NG // CHUNK       # 4 sim chunks per row tile

# Schraudolph integer exp, int16/bf16 variant: the bf16 bit pattern of
# exp(x) is approximately int16(A16*x + B16); bf16 output gets the DVE
# 2x reduce rate. B16 offset tuned for minimal loss bias.
SCH_A16 = float(2.0**23 / np.log(2.0) / 65536.0)
SCH_MUL = SCH_A16 * SCALE
SCH_ADD = float((16256.0 - 4.0) - SCH_A16 * SCALE)


def _build():
    nc = bacc.Bacc(num_devices=NCORES)

    x = nc.dram_tensor("x", [2, 128, KT, HM], F8, kind="ExternalInput")
    w0 = nc.dram_tensor("w0", [NT, 128, KT, 128], F8, kind="ExternalInput")
    w1 = nc.dram_tensor("w1", [NT, 128, KT, 128], F8, kind="ExternalInput")
    w2 = nc.dram_tensor("w2", [128, KT, 128], F8, kind="ExternalInput")
    ball = nc.dram_tensor("ball", [128, 2 * NT + 1], F32, kind="ExternalInput")
    out = nc.dram_tensor("out", [128, MT], F32, kind="ExternalOutput")
    ftout = nc.dram_tensor("ftout", [128, M], BF, kind="ExternalOutput")

    AF = mybir.ActivationFunctionType
    MULT = mybir.AluOpType.mult
    ADD = mybir.AluOpType.add
    DR = mybir.MatmulPerfMode.DoubleRow
    groups = [list(range(NCORES))]

    with tile.TileContext(nc) as tc:
        with (
            tc.tile_pool(name="singles", bufs=1) as singles,
            tc.tile_pool(name="esc", bufs=3) as esc,
            tc.tile_pool(name="pmm", bufs=4, space="PSUM") as pmm,
            tc.tile_pool(name="dram", bufs=1, space="DRAM") as dram,
        ):
            # ---- SBUF tiles ----
            w0s = singles.tile([128, NT, KT, 128], F8)
            w1s = singles.tile([128, NT, KT, 128], F8)
            wsl2 = singles.tile([128, KT, 128], F8)
            xt = singles.tile([128, 2, KT, HM], F8)
            balls = singles.tile([128, 2 * NT + 1], F32)
            h0 = singles.tile([128, NT, M], F8)
            h1 = singles.tile([128, NT, M], F8)
            eT = singles.tile([128, M], F32)
            sq = singles.tile([128, M], BF)
            rnorm = singles.tile([128, M], F32)
            rrec = singles.tile([128, M], F32)
            fT = singles.tile([128, M], BF)
            FT = singles.tile([128, NG], BF)
            ones = singles.tile([128, 128], BF)
            nbias = singles.tile([128, 1], F32)
            sums = singles.tile([128, MT, 2 * NCHUNK], F32)
            stot = singles.tile([128, MT], F32)

            # ---- input DMAs, first-use order across 4 posting queues ----
            def wslab(dst, src, lo, hi, eng):
                eng.dma_start(
                    out=dst[:, lo:hi, :, :],
                    in_=src[lo:hi].rearrange("t p k n -> p t k n"),
                )

            # The sync ring carries the collective-critical small DMAs plus
            # an early share of the input stream (all input items complete
            # by ~40us, well before cc_inA is needed); the scalar ring takes
            # the rest of the inputs. The gpsimd ring carries ONLY the
            # collectives so their triggers are never queued behind bulk
            # DMA issues.
            warm_in = dram.tile([128, 256], BF, name="warm_in")
            warm_out = dram.tile(
                [NCORES * 128, 256], BF, name="warm_out", addr_space="Shared"
            )
            nc.scalar.dma_start(out=warm_in, in_=x[1, :, 0:1, :].bitcast(BF))
            nc.gpsimd.collective_compute(
                "AllGather", mybir.AluOpType.bypass, replica_groups=groups,
                ins=[warm_in.opt()], outs=[warm_out.opt()],
            )

            nc.sync.dma_start(out=balls, in_=ball[:, :])
            nc.sync.dma_start(out=xt[:, 0, 0:4, :], in_=x[0, :, 0:4, :])
            wslab(w0s, w0, 0, 1, nc.scalar)
            nc.sync.dma_start(out=xt[:, 0, 4:8, :], in_=x[0, :, 4:8, :])
            nc.scalar.dma_start(out=xt[:, 0, 8:KT, :], in_=x[0, :, 8:KT, :])
            wslab(w0s, w0, 1, 5, nc.scalar)
            wslab(w0s, w0, 5, 10, nc.sync)
            wslab(w0s, w0, 10, NT, nc.scalar)
            wslab(w1s, w1, 0, 5, nc.sync)
            wslab(w1s, w1, 5, NT, nc.scalar)
            nc.sync.dma_start(out=xt[:, 1, :, :], in_=x[1, :, :, :])
            nc.scalar.dma_start(out=wsl2, in_=w2[:, :, :])

            nc.vector.memset(ones, 1.0)
            nc.vector.memset(nbias, -SCALE)

            def hidden_layer(src_ap, dst, wt, boff, func, hsl, extra=None):
                """fp8 DoubleRow layer on one 512-row half; 2 tn chains per
                [128,1024] PSUM tile (2 banks), 2 512-col ACT drains. extra:
                dict group_idx -> list of callbacks emitted after that
                group."""
                width = hsl.stop - hsl.start
                tpg = 1024 // width  # tn chains per [128,1024] psum tile
                for gi, tn0 in enumerate(range(0, NT, tpg)):
                    ps = pmm.tile([128, 1024], F32, tag="mm")
                    for j in range(tpg):
                        tn = tn0 + j
                        for tk in range(0, KT, 2):
                            nc.tensor.matmul(
                                ps[:, j * width : (j + 1) * width],
                                lhsT=wt[:, tn, tk : tk + 2, :],
                                rhs=src_ap(tk, hsl),
                                start=(tk == 0),
                                stop=(tk == KT - 2),
                                perf_mode=DR,
                            )
                    for j in range(tpg):
                        tn = tn0 + j
                        nc.scalar.activation(
                            out=dst[:, tn, hsl],
                            in_=ps[:, j * width : (j + 1) * width],
                            func=func,
                            bias=balls[:, boff + tn : boff + tn + 1],
                            scale=1.0,
                        )
                    if extra and gi in extra:
                        for cb in extra[gi]:
                            cb()

            def l2_norm(h, hsl):
                """L2 layer + normalize rows hsl -> fT[:, hsl] (bf16)."""
                width = hsl.stop - hsl.start
                ps2 = pmm.tile([128, 1024], F32, tag="mm")
                for tk in range(0, KT, 2):
                    nc.tensor.matmul(
                        ps2[:, 0:width],
                        lhsT=wsl2[:, tk : tk + 2, :],
                        rhs=h[:, tk : tk + 2, hsl],
                        start=(tk == 0),
                        stop=(tk == KT - 2),
                        perf_mode=DR,
                    )
                nc.scalar.activation(
                    out=eT[:, hsl], in_=ps2[:, 0:width], func=AF.Identity,
                    bias=balls[:, 2 * NT : 2 * NT + 1], scale=1.0,
                )
                nc.scalar.activation(
                    out=sq[:, hsl], in_=eT[:, hsl], func=AF.Square, scale=1.0,
                )
                nc.tensor.matmul(
                    ps2[:, width : 2 * width], lhsT=ones, rhs=sq[:, hsl],
                    start=True, stop=True,
                )
                nc.scalar.activation(
                    out=rnorm[:, hsl], in_=ps2[:, width : 2 * width],
                    func=AF.Sqrt, scale=1.0,
                )
                nc.vector.reciprocal_approx_fast(
                    out=rrec[:, hsl], in_=rnorm[:, hsl]
                )
                nc.vector.tensor_mul(fT[:, hsl], eT[:, hsl], rrec[:, hsl])

            def gather(hsl, ft_off, tag):
                """all-gather fT[:, hsl]; blocks land FT[:, ft_off:...]."""
                width = hsl.stop - hsl.start
                cc_in = dram.tile([128, width], BF, name=f"cc_in{tag}")
                cc_out = dram.tile(
                    [NCORES * 128, width], BF, name=f"cc_out{tag}",
                    addr_space="Shared",
                )
                nc.sync.dma_start(out=cc_in, in_=fT[:, hsl])
                nc.gpsimd.collective_compute(
                    "AllGather", mybir.AluOpType.bypass, replica_groups=groups,
                    ins=[cc_in.opt()], outs=[cc_out.opt()],
                )
                half = NCORES // 2
                nc.sync.dma_start(
                    out=FT[:, ft_off : ft_off + half * width],
                    in_=cc_out[0 : half * 128, :].rearrange(
                        "(r p) w -> p r w", r=half
                    ),
                )
                nc.sync.dma_start(
                    out=FT[:, ft_off + half * width : ft_off + NCORES * width],
                    in_=cc_out[half * 128 :, :].rearrange(
                        "(r p) w -> p r w", r=half
                    ),
                )

            def sim_chunk(c, m, eng):
                """sim rows m-tile x FT columns chunk c (2048 cols as two
                [128,1024] PSUM halves); exp-sum on eng."""
                lhs = fT[:, m * 128 : (m + 1) * 128]
                for h in range(2):
                    ps = pmm.tile([128, 1024], F32, tag="mm")
                    for q in range(2):
                        j0 = c * CHUNK + h * 1024 + q * 512
                        nc.tensor.matmul(
                            ps[:, q * 512 : (q + 1) * 512],
                            lhsT=lhs, rhs=FT[:, j0 : j0 + 512],
                            start=True, stop=True,
                        )
                    slot = sums[:, m, 2 * c + h : 2 * c + h + 1]
                    if eng == "dve":
                        sch = esc.tile([128, 1024], mybir.dt.int16, tag="sch")
                        nc.vector.tensor_scalar(
                            out=sch, in0=ps, scalar1=SCH_MUL, scalar2=SCH_ADD,
                            op0=MULT, op1=ADD,
                        )
                        nc.vector.reduce_sum(
                            slot, sch.bitcast(BF), axis=mybir.AxisListType.X,
                        )
                    else:
                        escr = esc.tile([128, 1024], BF, tag="escr")
                        nc.scalar.activation(
                            out=escr, in_=ps, func=AF.Exp, scale=SCALE,
                            bias=nbias, accum_out=slot,
                        )

            xsrc = lambda h: (lambda tk, hsl: xt[:, h, tk : tk + 2, :])
            hsrc = lambda hh: (lambda tk, hsl: hh[:, tk : tk + 2, hsl])

            # ---- first half: rows 0:512; gather A as two 64KB pieces so
            # the c0 columns land ~12us before c1 ----
            h0sl = slice(0, HM)
            h1sl = slice(HM, M)
            q3sl = slice(HM, HM + HM // 2)
            q4sl = slice(HM + HM // 2, M)
            hidden_layer(xsrc(0), h0, w0s, 0, AF.Relu, h0sl)
            hidden_layer(hsrc(h0), h1, w1s, NT, AF.Identity, h0sl)
            l2_norm(h1, h0sl)
            gather(slice(0, HM // 2), 0, "A1")
            gather(slice(HM // 2, HM), NG // 4, "A2")

            # ---- second half; L1 by 256-row quarters so gather Q3
            # launches ~20us before Q4 and the meshes overlap the
            # L1-q4 compute ----
            hidden_layer(xsrc(1), h0, w0s, 0, AF.Relu, h1sl)
            hidden_layer(hsrc(h0), h1, w1s, NT, AF.Identity, q3sl)
            l2_norm(h1, q3sl)
            gather(q3sl, NG // 2, "Q3")
            hidden_layer(hsrc(h0), h1, w1s, NT, AF.Identity, q4sl)
            l2_norm(h1, q4sl)
            gather(q4sl, 3 * NG // 4, "Q4")

            # ---- all 32 sim chunks, ordered by column availability; exp
            # split ACT ~20 / DVE ~12 so both engines finish together ----
            tail = [(c, m) for c in range(2) for m in range(MT)]
            tail += [(2, m) for m in range(MT)]
            tail += [(3, m) for m in range(MT)]
            for i, (c, m) in enumerate(tail):
                sim_chunk(c, m, "dve" if i % 8 in (2, 5, 7) else "act")

            nc.vector.reduce_sum(stot, sums, axis=mybir.AxisListType.X)
            nc.sync.dma_start(out=ftout[:, 0:HM], in_=fT[:, h0sl])
            nc.sync.dma_start(out=ftout[:, HM:M], in_=fT[:, h1sl])
            nc.sync.dma_start(out=out[:, :], in_=stot)

    nc.finalize()
    return nc


_NC_CACHE = None


def _get_nc():
    global _NC_CACHE
    if _NC_CACHE is None:
        _NC_CACHE = _build()
    return _NC_CACHE


def host_reduce(stot_rows, f):
    """Per-core loss sum from device exp-sums and bf16 features.

    stot_rows: [M] f64 row sums of exp((S-1)/T) incl. the self term.
    f: [M, E] f64 (bf16-rounded features, exactly what the matmul used).
    """
    dself = (f * f).sum(axis=1)
    dpos = (f[:HM] * f[HM:]).sum(axis=1)
    dpos = np.concatenate([dpos, dpos])
    sexcl = stot_rows - np.exp(SCALE * dself - SCALE)
    return (np.log(sexcl) + SCALE * (1.0 - dpos)).sum()


def _prep_w(W, ntiles):
    K = W.shape[0]
    kt = K // 128
    arr = W.reshape(kt, 128, ntiles, 128).transpose(2, 1, 0, 3)
    return np.ascontiguousarray(arr.astype(FP8))


def _prep_b(b, ntiles):
    return np.asarray(b, np.float32).reshape(ntiles, 128).T


def kernel(input1, input2, W0, b0, W1, b1, W2, b2):
    input1 = np.asarray(input1, np.float32)
    input2 = np.asarray(input2, np.float32)
    w0p = _prep_w(np.asarray(W0, np.float32), NT)
    w1p = _prep_w(np.asarray(W1, np.float32), NT)
    w2p = _prep_w(np.asarray(W2, np.float32), 1)[0]
    ballp = np.ascontiguousarray(
        np.concatenate(
            [_prep_b(b0, NT), _prep_b(b1, NT),
             np.asarray(b2, np.float32).reshape(128, 1)],
            axis=1,
        )
    )

    in_maps = []
    for r in range(NCORES):
        xr = np.concatenate(
            [input1[r * BS : (r + 1) * BS], input2[r * BS : (r + 1) * BS]],
            axis=0,
        )
        # [2, 128, KT, HM]: half-major, features on partitions
        xp = np.ascontiguousarray(
            xr.reshape(2, HM, KT, 128).transpose(0, 3, 2, 1).astype(FP8)
        )
        in_maps.append(
            {"x": xp, "w0": w0p, "w1": w1p, "w2": w2p, "ball": ballp}
        )

    nc = _get_nc()
    res = run_bass_kernel_spmd(
        nc,
        in_maps,
        core_ids=list(range(NCORES)),
        trace=bool(int(os.environ.get("KERNEL_TRACE", "0"))),
    )
    total = np.float64(0.0)
    for r in range(NCORES):
        stot = np.asarray(res.results[r]["out"], np.float64)  # [128, MT]
        ft = np.asarray(res.results[r]["ftout"]).astype(np.float64)  # [128, M]
        # stot[:, m] is rows m*128..(m+1)*128; flatten to [M]
        stot_rows = stot.T.reshape(M)
        f = ft.T  # [M, E]
        total += host_reduce(stot_rows, f)
    loss = np.float32(total / (2 * B))
    if res.exec_time_ns is not None:
        kernel.last_exec_time_ns = res.exec_time_ns
    return np.asarray(loss, np.float32)


kernel.last_exec_time_ns = None


# revision 14
# speedup vs baseline: 1.1447x; 1.0009x over previous
"""ContrastiveHead loss kernel for 8 Trainium2 NeuronCores.

Data-parallel shard of B across 8 cores; each core MLPs its 1024 rows
(transposed layout: features on partitions, rows on the free dim; fp8
DoubleRow matmuls), normalizes the [E=128, 1024] features, all-gathers
bf16 features, then computes its [1024, 8192] block of the similarity
matrix and the masked logsumexp sums. Per-row exp-sums plus the raw
bf16 features go back to the host, which recomputes the self/pos
similarities exactly and finishes the loss in f64.

Schedule (all on one PE stream, emission order == execution order):
- biases first, then x/w0/w1/w2 DMAs in first-use order, coarse slabs
  across 4 posting queues.
- warmup 2KB AllGather right away so the CC engine's ~10us first-use
  startup cost is paid during the DMA ramp.
- L0h0 -> L1h0 -> L2h0+norm -> gather A (rows 0:512, Shared output)
  issued at ~1/2 of the MLP, fully hidden under the h1 half.
- L0h1 -> L1h1 with the 8 sim chunks over gather-A columns x h0 rows
  interleaved between groups; their exp runs as Schraudolph int16
  (bf16-bitcast) on the otherwise idle Vector engine, keeping the
  Scalar engine on the Identity/Sqrt table until the tail.
- L2h1+norm -> gathers Q3/Q4 (rows 512:768, 768:1024).
- tail: remaining 24 sim chunks; exp split ACT (Exp+accum_out, ~15) /
  DVE (Schraudolph, ~9) to finish together.
- logsumexp via the constant bound max=1 (normalized rows): lse = 1/T +
  log(sum_j exp((S_ij-1)/T)); self term excluded on the host using the
  shipped bf16 features (exact same values the sim matmul consumed).
"""

import os
import sys

for _p in ("/opt/trn_rl_repo",):
    if os.path.isdir(_p) and _p not in sys.path:
        sys.path.append(_p)

import ml_dtypes
import numpy as np

import concourse.bass as bass
import concourse.mybir as mybir
import concourse.tile as tile
from concourse import bacc
from concourse.bass_utils import run_bass_kernel_spmd

BF16 = ml_dtypes.bfloat16
F32 = mybir.dt.float32
BF = mybir.dt.bfloat16
F8 = mybir.dt.float8e4
FP8 = mybir.dt.np(F8)

B, D, H, E = 4096, 2048, 2048, 128
T = 0.07
SCALE = float(1.0 / T)
NCORES = 8
BS = B // NCORES          # rows per view per core (512)
M = 2 * BS                # local feature rows (1024)
HM = M // 2               # rows per pipeline half (512)
KT = D // 128             # 16 contraction tiles for D/H
NT = H // 128             # 16 output-feature tiles for hidden layers
MT = M // 128              # 8 local row tiles
NG = NCORES * M            # 8192 gathered rows
CHUNK = 2048               # sim free-dim chunk (4-bank PSUM tile)
NCHUNK = NG // CHUNK       # 4 sim chunks per row tile

# Schraudolph integer exp, int16/bf16 variant: the bf16 bit pattern of
# exp(x) is approximately int16(A16*x + B16); bf16 output gets the DVE
# 2x reduce rate. B16 offset tuned for minimal loss bias.
SCH_A16 = float(2.0**23 / np.log(2.0) / 65536.0)
SCH_MUL = SCH_A16 * SCALE
SCH_ADD = float((16256.0 - 4.0) - SCH_A16 * SCALE)


def _build():
    nc = bacc.Bacc(num_devices=NCORES)

    x = nc.dram_tensor("x", [2, 128, KT, HM], F8, kind="ExternalInput")
    w0 = nc.dram_tensor("w0", [NT, 128, KT, 128], F8, kind="ExternalInput")
    w1 = nc.dram_tensor("w1", [NT, 128, KT, 128], F8, kind="ExternalInput")
    w2 = nc.dram_tensor("w2", [128, KT, 128], F8, kind="ExternalInput")
    ball = nc.dram_tensor("ball", [128, 2 * NT + 1], F32, kind="ExternalInput")
    out = nc.dram_tensor("out", [128, MT], F32, kind="ExternalOutput")
    ftout = nc.dram_tensor("ftout", [128, M], BF, kind="ExternalOutput")

    AF = mybir.ActivationFunctionType
    MULT = mybir.AluOpType.mult
    ADD = mybir.AluOpType.add
    DR = mybir.MatmulPerfMode.DoubleRow
    groups = [list(range(NCORES))]

    with tile.TileContext(nc) as tc:
        with (
            tc.tile_pool(name="singles", bufs=1) as singles,
            tc.tile_pool(name="esc", bufs=3) as esc,
            tc.tile_pool(name="pmm", bufs=4, space="PSUM") as pmm,
            tc.tile_pool(name="dram", bufs=1, space="DRAM") as dram,
        ):
            # ---- SBUF tiles ----
            w0s = singles.tile([128, NT, KT, 128], F8)
            w1s = singles.tile([128, NT, KT, 128], F8)
            wsl2 = singles.tile([128, KT, 128], F8)
            xt = singles.tile([128, 2, KT, HM], F8)
            balls = singles.tile([128, 2 * NT + 1], F32)
            h0 = singles.tile([128, NT, M], F8)
            h1 = singles.tile([128, NT, M], F8)
            eT = singles.tile([128, M], F32)
            sq = singles.tile([128, M], BF)
            rnorm = singles.tile([128, M], F32)
            rrec = singles.tile([128, M], F32)
            fT = singles.tile([128, M], BF)
            FT = singles.tile([128, NG], BF)
            ones = singles.tile([128, 128], BF)
            nbias = singles.tile([128, 1], F32)
            sums = singles.tile([128, MT, 2 * NCHUNK], F32)
            stot = singles.tile([128, MT], F32)

            # ---- input DMAs, first-use order across 4 posting queues ----
            def wslab(dst, src, lo, hi, eng):
                eng.dma_start(
                    out=dst[:, lo:hi, :, :],
                    in_=src[lo:hi].rearrange("t p k n -> p t k n"),
                )

            # The sync ring carries the collective-critical small DMAs plus
            # an early share of the input stream (all input items complete
            # by ~40us, well before cc_inA is needed); the scalar ring takes
            # the rest of the inputs. The gpsimd ring carries ONLY the
            # collectives so their triggers are never queued behind bulk
            # DMA issues.
            warm_in = dram.tile([128, 256], BF, name="warm_in")
            warm_out = dram.tile(
                [NCORES * 128, 256], BF, name="warm_out", addr_space="Shared"
            )
            nc.scalar.dma_start(out=warm_in, in_=x[1, :, 0:1, :].bitcast(BF))
            nc.gpsimd.collective_compute(
                "AllGather", mybir.AluOpType.bypass, replica_groups=groups,
                ins=[warm_in.opt()], outs=[warm_out.opt()],
            )

            nc.sync.dma_start(out=balls, in_=ball[:, :])
            nc.sync.dma_start(out=xt[:, 0, 0:4, :], in_=x[0, :, 0:4, :])
            wslab(w0s, w0, 0, 1, nc.scalar)
            nc.sync.dma_start(out=xt[:, 0, 4:8, :], in_=x[0, :, 4:8, :])
            wslab(w0s, w0, 1, 3, nc.scalar)
            nc.scalar.dma_start(out=xt[:, 0, 8:KT, :], in_=x[0, :, 8:KT, :])
            wslab(w0s, w0, 3, 5, nc.scalar)
            wslab(w0s, w0, 5, 10, nc.sync)
            wslab(w0s, w0, 10, NT, nc.scalar)
            wslab(w1s, w1, 0, 5, nc.sync)
            wslab(w1s, w1, 5, NT, nc.scalar)
            nc.sync.dma_start(out=xt[:, 1, :, :], in_=x[1, :, :, :])
            nc.scalar.dma_start(out=wsl2, in_=w2[:, :, :])

            nc.vector.memset(ones, 1.0)
            nc.vector.memset(nbias, -SCALE)

            def hidden_layer(src_ap, dst, wt, boff, func, hsl, extra=None):
                """fp8 DoubleRow layer on one 512-row half; 2 tn chains per
                [128,1024] PSUM tile (2 banks), 2 512-col ACT drains. extra:
                dict group_idx -> list of callbacks emitted after that
                group."""
                width = hsl.stop - hsl.start
                tpg = 1024 // width  # tn chains per [128,1024] psum tile
                for gi, tn0 in enumerate(range(0, NT, tpg)):
                    ps = pmm.tile([128, 1024], F32, tag="mm")
                    for j in range(tpg):
                        tn = tn0 + j
                        for tk in range(0, KT, 2):
                            nc.tensor.matmul(
                                ps[:, j * width : (j + 1) * width],
                                lhsT=wt[:, tn, tk : tk + 2, :],
                                rhs=src_ap(tk, hsl),
                                start=(tk == 0),
                                stop=(tk == KT - 2),
                                perf_mode=DR,
                            )
                    for j in range(tpg):
                        tn = tn0 + j
                        nc.scalar.activation(
                            out=dst[:, tn, hsl],
                            in_=ps[:, j * width : (j + 1) * width],
                            func=func,
                            bias=balls[:, boff + tn : boff + tn + 1],
                            scale=1.0,
                        )
                    if extra and gi in extra:
                        for cb in extra[gi]:
                            cb()

            def l2_norm(h, hsl):
                """L2 layer + normalize rows hsl -> fT[:, hsl] (bf16)."""
                width = hsl.stop - hsl.start
                ps2 = pmm.tile([128, 1024], F32, tag="mm")
                for tk in range(0, KT, 2):
                    nc.tensor.matmul(
                        ps2[:, 0:width],
                        lhsT=wsl2[:, tk : tk + 2, :],
                        rhs=h[:, tk : tk + 2, hsl],
                        start=(tk == 0),
                        stop=(tk == KT - 2),
                        perf_mode=DR,
                    )
                nc.scalar.activation(
                    out=eT[:, hsl], in_=ps2[:, 0:width], func=AF.Identity,
                    bias=balls[:, 2 * NT : 2 * NT + 1], scale=1.0,
                )
                nc.scalar.activation(
                    out=sq[:, hsl], in_=eT[:, hsl], func=AF.Square, scale=1.0,
                )
                nc.tensor.matmul(
                    ps2[:, width : 2 * width], lhsT=ones, rhs=sq[:, hsl],
                    start=True, stop=True,
                )
                nc.scalar.activation(
                    out=rnorm[:, hsl], in_=ps2[:, width : 2 * width],
                    func=AF.Sqrt, scale=1.0,
                )
                nc.vector.reciprocal_approx_fast(
                    out=rrec[:, hsl], in_=rnorm[:, hsl]
                )
                nc.vector.tensor_mul(fT[:, hsl], eT[:, hsl], rrec[:, hsl])

            def gather(hsl, ft_off, tag):
                """all-gather fT[:, hsl]; blocks land FT[:, ft_off:...]."""
                width = hsl.stop - hsl.start
                cc_in = dram.tile([128, width], BF, name=f"cc_in{tag}")
                cc_out = dram.tile(
                    [NCORES * 128, width], BF, name=f"cc_out{tag}",
                    addr_space="Shared",
                )
                nc.sync.dma_start(out=cc_in, in_=fT[:, hsl])
                nc.gpsimd.collective_compute(
                    "AllGather", mybir.AluOpType.bypass, replica_groups=groups,
                    ins=[cc_in.opt()], outs=[cc_out.opt()],
                )
                half = NCORES // 2
                nc.sync.dma_start(
                    out=FT[:, ft_off : ft_off + half * width],
                    in_=cc_out[0 : half * 128, :].rearrange(
                        "(r p) w -> p r w", r=half
                    ),
                )
                nc.sync.dma_start(
                    out=FT[:, ft_off + half * width : ft_off + NCORES * width],
                    in_=cc_out[half * 128 :, :].rearrange(
                        "(r p) w -> p r w", r=half
                    ),
                )

            def sim_chunk(c, m, eng):
                """sim rows m-tile x FT columns chunk c (2048 cols as two
                [128,1024] PSUM halves); exp-sum on eng."""
                lhs = fT[:, m * 128 : (m + 1) * 128]
                for h in range(2):
                    ps = pmm.tile([128, 1024], F32, tag="mm")
                    for q in range(2):
                        j0 = c * CHUNK + h * 1024 + q * 512
                        nc.tensor.matmul(
                            ps[:, q * 512 : (q + 1) * 512],
                            lhsT=lhs, rhs=FT[:, j0 : j0 + 512],
                            start=True, stop=True,
                        )
                    slot = sums[:, m, 2 * c + h : 2 * c + h + 1]
                    if eng == "dve":
                        sch = esc.tile([128, 1024], mybir.dt.int16, tag="sch")
                        nc.vector.tensor_scalar(
                            out=sch, in0=ps, scalar1=SCH_MUL, scalar2=SCH_ADD,
                            op0=MULT, op1=ADD,
                        )
                        nc.vector.reduce_sum(
                            slot, sch.bitcast(BF), axis=mybir.AxisListType.X,
                        )
                    else:
                        escr = esc.tile([128, 1024], BF, tag="escr")
                        nc.scalar.activation(
                            out=escr, in_=ps, func=AF.Exp, scale=SCALE,
                            bias=nbias, accum_out=slot,
                        )

            xsrc = lambda h, base: (
                lambda tk, hsl: xt[
                    :, h, tk : tk + 2, hsl.start - base : hsl.stop - base
                ]
            )
            hsrc = lambda hh: (lambda tk, hsl: hh[:, tk : tk + 2, hsl])

            # ---- first half: rows 0:512; gather A as two 64KB pieces so
            # the c0 columns land ~12us before c1 ----
            h0sl = slice(0, HM)
            h1sl = slice(HM, M)
            q3sl = slice(HM, HM + HM // 2)
            q4sl = slice(HM + HM // 2, M)
            q1sl = slice(0, HM // 2)
            q2sl = slice(HM // 2, HM)
            hidden_layer(xsrc(0, 0), h0, w0s, 0, AF.Relu, q1sl)
            hidden_layer(xsrc(0, 0), h0, w0s, 0, AF.Relu, q2sl)
            hidden_layer(hsrc(h0), h1, w1s, NT, AF.Identity, q1sl)
            hidden_layer(hsrc(h0), h1, w1s, NT, AF.Identity, q2sl)
            l2_norm(h1, h0sl)
            gather(q1sl, 0, "A1")
            gather(q2sl, NG // 4, "A2")
            # (b) filler gathers: keep the CC engine hot between A2 and Q3
            # (a cold CC restart costs ~28us vs ~9-18 when kept busy)
            for fi in range(2):
                fill_out = dram.tile(
                    [NCORES * 128, 256], BF, name=f"fill_out{fi}",
                    addr_space="Shared",
                )
                nc.gpsimd.collective_compute(
                    "AllGather", mybir.AluOpType.bypass,
                    replica_groups=groups,
                    ins=[warm_in.opt()], outs=[fill_out.opt()],
                )

            # ---- second half; L1 by 256-row quarters so gather Q3
            # launches ~20us before Q4 and the meshes overlap the
            # L1-q4 compute ----
            hidden_layer(xsrc(1, HM), h0, w0s, 0, AF.Relu, q3sl)
            hidden_layer(xsrc(1, HM), h0, w0s, 0, AF.Relu, q4sl)
            hidden_layer(hsrc(h0), h1, w1s, NT, AF.Identity, q3sl)
            l2_norm(h1, q3sl)
            gather(q3sl, NG // 2, "Q3")
            hidden_layer(hsrc(h0), h1, w1s, NT, AF.Identity, q4sl)
            l2_norm(h1, q4sl)
            gather(q4sl, 3 * NG // 4, "Q4")

            # ---- all 32 sim chunks, ordered by column availability; exp
            # split ACT ~20 / DVE ~12 so both engines finish together ----
            tail = [(c, m) for c in range(2) for m in range(MT)]
            tail += [(2, m) for m in range(MT)]
            tail += [(3, m) for m in range(MT)]
            for i, (c, m) in enumerate(tail):
                sim_chunk(c, m, "dve" if i % 8 in (2, 5, 7) else "act")

            nc.vector.reduce_sum(stot, sums, axis=mybir.AxisListType.X)
            nc.sync.dma_start(out=ftout[:, 0:HM], in_=fT[:, h0sl])
            nc.sync.dma_start(out=ftout[:, HM:M], in_=fT[:, h1sl])
            nc.sync.dma_start(out=out[:, :], in_=stot)

    nc.finalize()
    return nc


_NC_CACHE = None


def _get_nc():
    global _NC_CACHE
    if _NC_CACHE is None:
        _NC_CACHE = _build()
    return _NC_CACHE


def host_reduce(stot_rows, f):
    """Per-core loss sum from device exp-sums and bf16 features.

    stot_rows: [M] f64 row sums of exp((S-1)/T) incl. the self term.
    f: [M, E] f64 (bf16-rounded features, exactly what the matmul used).
    """
    dself = (f * f).sum(axis=1)
    dpos = (f[:HM] * f[HM:]).sum(axis=1)
    dpos = np.concatenate([dpos, dpos])
    sexcl = stot_rows - np.exp(SCALE * dself - SCALE)
    return (np.log(sexcl) + SCALE * (1.0 - dpos)).sum()


def _prep_w(W, ntiles):
    K = W.shape[0]
    kt = K // 128
    arr = W.reshape(kt, 128, ntiles, 128).transpose(2, 1, 0, 3)
    return np.ascontiguousarray(arr.astype(FP8))


def _prep_b(b, ntiles):
    return np.asarray(b, np.float32).reshape(ntiles, 128).T


def kernel(input1, input2, W0, b0, W1, b1, W2, b2):
    input1 = np.asarray(input1, np.float32)
    input2 = np.asarray(input2, np.float32)
    w0p = _prep_w(np.asarray(W0, np.float32), NT)
    w1p = _prep_w(np.asarray(W1, np.float32), NT)
    w2p = _prep_w(np.asarray(W2, np.float32), 1)[0]
    ballp = np.ascontiguousarray(
        np.concatenate(
            [_prep_b(b0, NT), _prep_b(b1, NT),
             np.asarray(b2, np.float32).reshape(128, 1)],
            axis=1,
        )
    )

    in_maps = []
    for r in range(NCORES):
        xr = np.concatenate(
            [input1[r * BS : (r + 1) * BS], input2[r * BS : (r + 1) * BS]],
            axis=0,
        )
        # [2, 128, KT, HM]: half-major, features on partitions
        xp = np.ascontiguousarray(
            xr.reshape(2, HM, KT, 128).transpose(0, 3, 2, 1).astype(FP8)
        )
        in_maps.append(
            {"x": xp, "w0": w0p, "w1": w1p, "w2": w2p, "ball": ballp}
        )

    nc = _get_nc()
    res = run_bass_kernel_spmd(
        nc,
        in_maps,
        core_ids=list(range(NCORES)),
        trace=bool(int(os.environ.get("KERNEL_TRACE", "0"))),
    )
    total = np.float64(0.0)
    for r in range(NCORES):
        stot = np.asarray(res.results[r]["out"], np.float64)  # [128, MT]
        ft = np.asarray(res.results[r]["ftout"]).astype(np.float64)  # [128, M]
        # stot[:, m] is rows m*128..(m+1)*128; flatten to [M]
        stot_rows = stot.T.reshape(M)
        f = ft.T  # [M, E]
        total += host_reduce(stot_rows, f)
    loss = np.float32(total / (2 * B))
    if res.exec_time_ns is not None:
        kernel.last_exec_time_ns = res.exec_time_ns
    return np.asarray(loss, np.float32)


kernel.last_exec_time_ns = None
